# revision 1
# baseline (speedup 1.0000x reference)
"""Trainium2 Bass kernel for nn_CrossAttentionLayer (B=16, S=2048, E=1024, H=4).

Sharding: data-parallel over batch B across 8 NeuronCores (2 batches/core);
projection weights replicated.

Math reduction: the reference's final einsum("bqk,bke->be", avg_w, attn_output)
collapses to
    out[b] = (s_vec @ ctx_full) @ out_w.T + S * out_b
where s_vec[k] = mean_h sum_q softmax_w_h[q, k] and ctx_full [S, E] is the
per-head attention context — the full [S, E] output projection is never built.

Per-core pipeline (all matmuls float32r: full PE rate at N=512, ~1e-4 rel err;
x^T stays resident in SBUF and q/k/v are generated per head on the fly, so the
only DRAM round-trip is the ctx^T spill):
  per (batch, head):
    1. qT,kT [dh, s] and v [s, dh] generated from resident x^T and streamed
       per-head weight slices; 1/sqrt(DH) and biases folded into the ACT
       Identity copyback (ACT, not DVE, to keep DVE free for softmax sums).
    2. per sq-quarter: scores^T = kT.T @ qT (PSUM); ACT exp with key-padding
       mask as per-partition bias (masked keys -> exp 0); softmax denominators
       l via ones-column matmul; ctx^T += v.T @ expT; l broadcast via K=1
       ones-row matmul + fast reciprocal; per-key colsums c (-> s_vec) via
       fused DVE multiply+accumulate.
  3. u = s_vec @ ctx_full via fused DVE multiply+accumulate over spilled ctx^T,
     then out[b] = u @ out_w.T + S*out_b as an M=1 matmul chain.
"""
import numpy as np
from contextlib import ExitStack

B, S, E, H = 16, 2048, 1024, 4
DH = E // H          # 256
NCORES = 8
BL = B // NCORES     # batches per core
NKT = S // 128       # sk tiles
NET = E // 128       # e tiles
QTR = 512            # sq chunk


def _build_nc(repeat=1):
    import concourse.bass as bass
    import concourse.tile as tile
    from concourse import bacc, mybir

    F32 = mybir.dt.float32
    F32R = mybir.dt.float32r
    AOp = mybir.AluOpType
    Act = mybir.ActivationFunctionType

    nc = bacc.Bacc()

    xT = nc.dram_tensor("xT", [BL, E, S], F32R, kind="ExternalInput")
    wt = nc.dram_tensor("wt", [E, 3 * E], F32R, kind="ExternalInput")
    owt = nc.dram_tensor("owt", [E, E], F32R, kind="ExternalInput")
    bqk = nc.dram_tensor("bqk", [128, 16], F32, kind="ExternalInput")
    bv = nc.dram_tensor("bv", [1, E], F32, kind="ExternalInput")
    maskb = nc.dram_tensor("maskb", [BL, 128, NKT], F32, kind="ExternalInput")
    outb = nc.dram_tensor("outb", [1, E], F32, kind="ExternalInput")
    ones_c = nc.dram_tensor("ones_c", [128, 1], F32R, kind="ExternalInput")
    ones_r = nc.dram_tensor("ones_r", [1, 128], F32R, kind="ExternalInput")
    out_o = nc.dram_tensor("out_o", [BL, E], F32, kind="ExternalOutput")

    ctxT_d = nc.dram_tensor("ctxT_d", [BL, E, S], mybir.dt.bfloat16)
    s_row_d = nc.dram_tensor("s_row_d", [BL, S], F32)

    wt_t = wt.rearrange("(eo p) j -> p eo j", p=128)    # [128, 8, 3072]
    owt_t = owt.rearrange("(eo p) j -> p eo j", p=128)  # [128, 8, 1024]

    def bcast_ap(dram_ap, n):
        # read the same n contiguous DRAM elements into all 128 partitions
        return bass.AP(tensor=dram_ap.tensor, offset=dram_ap.offset,
                       ap=[[0, 128], [1, n]])

    with tile.TileContext(nc) as tc, ExitStack() as ctx:
        const = ctx.enter_context(tc.tile_pool(name="const", bufs=1))

        bqk_sb = const.tile([128, 16], F32)
        nc.sync.dma_start(bqk_sb[:], bqk[:, :])
        maskb_sb = const.tile([128, BL * NKT], F32)
        for b in range(BL):
            nc.sync.dma_start(maskb_sb[:, b * NKT:(b + 1) * NKT], maskb[b, :, :])
        bv_bc = const.tile([128, E], F32)
        nc.sync.dma_start(bv_bc[:], bcast_ap(bv[0, :], E))
        outb_sb = const.tile([1, E], F32)
        nc.sync.dma_start(outb_sb[:], outb[:, :])
        ones_col = const.tile([128, 1], F32R)
        nc.sync.dma_start(ones_col[:], ones_c[:, :])
        ones_row = const.tile([1, 128], F32R)
        nc.sync.dma_start(ones_row[:], ones_r[:, :])

        def final_phase(b, fin, finc, psF, wide=False):
            # final reduction for batch b, emitted with pools owned by the
            # following batch's scope so both can stay allocated concurrently
            if True:
                owt_res = None
                if wide:
                    owt_res = fin.tile([128, 8, E], F32R, tag="owt_res")
                    nc.sync.dma_start(owt_res[:], owt_t[:, :, :])
                s_bc = fin.tile([128, S], F32, tag="s_bc")
                nc.sync.dma_start(s_bc[:], bcast_ap(s_row_d[b, :], S))

                u_cols = [fin.tile([128, NET], F32, tag=f"u_col{i}",
                                   name=f"u_col{i}") for i in range(2)]
                dummy2 = fin.tile([128, S // 2], mybir.dt.bfloat16,
                                  tag="dummy2")  # reused across both halves
                for et in range(NET):
                    for i in range(2):
                        ks = slice(i * (S // 2), (i + 1) * (S // 2))
                        ctx_t = finc.tile([128, S // 2], mybir.dt.bfloat16,
                                          tag="ctx_t", name=f"ctx_t{et}_{i}")
                        nc.sync.dma_start(
                            ctx_t[:], ctxT_d[b, et * 128:(et + 1) * 128, ks])
                        nc.vector.scalar_tensor_tensor(
                            out=dummy2[:], in0=ctx_t[:], scalar=1.0,
                            in1=s_bc[:, ks],
                            op0=AOp.mult, op1=AOp.mult,
                            accum_out=u_cols[i][:, et:et + 1])
                u_r = fin.tile([128, NET], F32R, tag="u_r")
                nc.vector.tensor_add(u_r[:], u_cols[0][:], u_cols[1][:])

                for fc in range(2):
                    cs = slice(fc * 512, (fc + 1) * 512)
                    psumF = psF.tile([1, 512], F32, tag="psumF",
                                     name=f"psumF{fc}")
                    for et in range(NET):
                        if wide:
                            owt_et = owt_res[:, et, cs]
                        else:
                            owt_t_sb = fin.tile([128, 512], F32R, tag="owt_et",
                                                name=f"owt{et}_{fc}")
                            nc.sync.dma_start(owt_t_sb[:], owt_t[:, et, cs])
                            owt_et = owt_t_sb[:]
                        nc.tensor.matmul(psumF[:], u_r[:, et:et + 1],
                                         owt_et,
                                         start=(et == 0),
                                         stop=(et == NET - 1))
                    out_row = fin.tile([1, 512], F32, tag="out_row",
                                       name=f"out_row{fc}")
                    nc.vector.tensor_add(out_row[:], psumF[:],
                                         outb_sb[:, cs])
                    nc.sync.dma_start(out_o[b:b + 1, cs], out_row[:])

        for rep in range(repeat):
          for b in range(BL):
            with tc.tile_pool(name=f"xp{b}r{rep}", bufs=1) as xp, \
                 tc.tile_pool(name=f"wp{b}r{rep}", bufs=1) as wp, \
                 tc.tile_pool(name=f"hd{b}r{rep}", bufs=1) as head, \
                 tc.tile_pool(name=f"ep{b}r{rep}", bufs=1) as epool, \
                 tc.tile_pool(name=f"sm{b}r{rep}", bufs=2) as small, \
                 tc.tile_pool(name=f"cx{b}r{rep}", bufs=1) as ctxp, \
                 tc.tile_pool(name=f"ac{b}r{rep}", bufs=1) as acc, \
                 tc.tile_pool(name=f"psB{b}r{rep}", bufs=1, space="PSUM") as psB, \
                 tc.tile_pool(name=f"psS{b}r{rep}", bufs=4, space="PSUM") as psS, \
                 tc.tile_pool(name=f"fn{b}r{rep}", bufs=1) as fin, \
                 tc.tile_pool(name=f"fc{b}r{rep}", bufs=1) as finc, \
                 tc.tile_pool(name=f"psF{b}r{rep}", bufs=1, space="PSUM") as psF:
                if b > 0 or rep > 0:
                    final_phase(b - 1 if b > 0 else BL - 1, fin, finc, psF)
                s_acc = acc.tile([128, NKT], F32, tag="s_acc")
                nc.vector.memset(s_acc[:], 0.0)

                def load_head_w(h):
                    wk_sb = wp.tile([128, 8, DH], F32R, tag="wk", name="wk")
                    nc.sync.dma_start(
                        wk_sb[:], wt_t[:, :, E + h * DH:E + (h + 1) * DH])
                    wv_sb = wp.tile([128, 8, DH], F32R, tag="wv", name="wv")
                    nc.sync.dma_start(
                        wv_sb[:], wt_t[:, :, 2 * E + h * DH:2 * E + (h + 1) * DH])
                    wq_sb = wp.tile([128, 8, DH], F32R, tag="wq", name="wq")
                    nc.sync.dma_start(
                        wq_sb[:], wt_t[:, :, h * DH:(h + 1) * DH])
                    return wq_sb, wk_sb, wv_sb

                wk0_sb = wp.tile([128, 8, DH], F32R, tag="wk", name="wk")
                nc.sync.dma_start(wk0_sb[:], wt_t[:, :, E:E + DH])
                xT_sb = xp.tile([128, 8, S], F32R, tag="xT")
                xr = xT[b].rearrange("(eo p) s -> p eo s", p=128)
                nc.sync.dma_start(xT_sb[:, :, 0:512], xr[:, :, 0:512])
                wv0_sb = wp.tile([128, 8, DH], F32R, tag="wv", name="wv")
                nc.sync.dma_start(wv0_sb[:], wt_t[:, :, 2 * E:2 * E + DH])
                for xc in range(1, 4):
                    nc.sync.dma_start(
                        xT_sb[:, :, xc * 512:(xc + 1) * 512],
                        xr[:, :, xc * 512:(xc + 1) * 512])
                wq0_sb = wp.tile([128, 8, DH], F32R, tag="wq", name="wq")
                nc.sync.dma_start(wq0_sb[:], wt_t[:, :, 0:DH])
                w_next = (wq0_sb, wk0_sb, wv0_sb)
                for h in range(H):
                    wq_sb, wk_sb, wv_sb = w_next
                    if h + 1 < H:
                        w_next = load_head_w(h + 1)

                    # ---- generate kT [128, 2, S] and v [128, 16, DH],
                    #      interleaved by xT s-chunk so work starts as soon
                    #      as the first chunk lands ----
                    kTh = head.tile([128, 2, S], F32R, tag="kTh")
                    vh = head.tile([128, NKT, DH], F32R, tag="vh")
                    for sc in range(4):
                        for jt2 in range(2):
                            jcol = 8 + 2 * h + jt2
                            gq = psS.tile([128, 512], F32, tag="sa",
                                          name="gk")
                            for eo in range(8):
                                nc.tensor.matmul(
                                    gq[:],
                                    wk_sb[:, eo, jt2 * 128:(jt2 + 1) * 128],
                                    xT_sb[:, eo, sc * 512:(sc + 1) * 512],
                                    start=(eo == 0), stop=(eo == 7))
                            nc.scalar.activation(
                                kTh[:, jt2, sc * 512:(sc + 1) * 512], gq[:],
                                Act.Identity,
                                bias=bqk_sb[:, jcol:jcol + 1],
                                scale=1.0)
                        for st in range(4 * sc, 4 * sc + 4):
                            gv = psS.tile([128, DH], F32, tag="sa", name="gv")
                            for eo in range(8):
                                nc.tensor.matmul(
                                    gv[:],
                                    xT_sb[:, eo, st * 128:(st + 1) * 128],
                                    wv_sb[:, eo, :],
                                    start=(eo == 0), stop=(eo == 7))
                            nc.vector.tensor_add(
                                vh[:, st, :], gv[:],
                                bv_bc[:, h * DH:(h + 1) * DH])

                    # ---- attention over sq quarters ----
                    ctx_sbs = [ctxp.tile([128, S], mybir.dt.bfloat16,
                                         tag=f"ctxsb{d_}",
                                         name=f"ctxsb{d_}") for d_ in range(2)]
                    for qtr in range(S // QTR):
                        q0 = qtr * QTR
                        # generate this quarter's qT [128, 2, QTR]
                        qTh = head.tile([128, 2, QTR], F32R, tag=f"qTh{qtr % 2}",
                                        name="qTh")
                        for jt2 in range(2):
                            jcol = 2 * h + jt2
                            gq = psS.tile([128, 512], F32, tag="sa", name="gq")
                            for eo in range(8):
                                nc.tensor.matmul(
                                    gq[:],
                                    wq_sb[:, eo, jt2 * 128:(jt2 + 1) * 128],
                                    xT_sb[:, eo, q0:q0 + QTR],
                                    start=(eo == 0), stop=(eo == 7))
                            nc.scalar.activation(
                                qTh[:, jt2, :], gq[:],
                                Act.Identity,
                                bias=bqk_sb[:, jcol:jcol + 1],
                                scale=1.0 / 16.0)
                        psumL = psB.tile([1, QTR], F32, tag="psumLR",
                                         name="psumL")
                        psumC0 = psB.tile([128, QTR], F32, tag="pvc0")
                        psumC1 = psB.tile([128, QTR], F32, tag="pvc1")
                        expts = []
                        for kt in range(NKT):
                            sa = psS.tile([128, QTR], F32, tag="sa", name="sa")
                            for do in range(2):
                                nc.tensor.matmul(
                                    sa[:],
                                    kTh[:, do, kt * 128:(kt + 1) * 128],
                                    qTh[:, do, :],
                                    start=(do == 0), stop=(do == 1))
                            ettag = (f"et{kt}q{qtr % 2}" if kt < 3
                                     else f"et{kt}")
                            et = epool.tile([128, QTR], F32R, tag=ettag,
                                            name=f"et{kt}")
                            nc.scalar.activation(
                                et[:], sa[:], Act.Exp,
                                bias=maskb_sb[:, b * NKT + kt:b * NKT + kt + 1],
                                scale=1.0)
                            expts.append(et)
                            nc.tensor.matmul(
                                psumL[:], ones_col[:], et[:],
                                start=(kt == 0), stop=(kt == NKT - 1))
                            nc.tensor.matmul(
                                psumC0[:], vh[:, kt, 0:128], et[:],
                                start=(kt == 0), stop=(kt == NKT - 1))
                            nc.tensor.matmul(
                                psumC1[:], vh[:, kt, 128:256], et[:],
                                start=(kt == 0), stop=(kt == NKT - 1))
                        l_row = ctxp.tile([1, QTR], F32R, tag="l_row")
                        nc.vector.tensor_copy(l_row[:], psumL[:])
                        psumR = psB.tile([128, QTR], F32, tag="psumLR",
                                         name="psumR")
                        nc.tensor.matmul(psumR[:], ones_row[:], l_row[:],
                                         start=True, stop=True)
                        r_bc = ctxp.tile([128, QTR], F32, tag="r_bc")
                        nc.vector.reciprocal_approx_fast(r_bc[:], psumR[:])

                        c_half = ctxp.tile([128, NKT], F32, tag="c_half")
                        dummy = ctxp.tile([128, QTR], mybir.dt.bfloat16,
                                          tag="dummy")
                        for kt in range(NKT):
                            nc.vector.scalar_tensor_tensor(
                                out=dummy[:], in0=expts[kt][:].bitcast(F32),
                                scalar=1.0 / H, in1=r_bc[:],
                                op0=AOp.mult, op1=AOp.mult,
                                accum_out=c_half[:, kt:kt + 1])
                        nc.vector.tensor_add(s_acc[:], s_acc[:], c_half[:])

                        for dhalf, psC in ((0, psumC0), (1, psumC1)):
                            nc.vector.scalar_tensor_tensor(
                                out=ctx_sbs[dhalf][:, q0:q0 + QTR], in0=psC[:],
                                scalar=1.0,
                                in1=r_bc[:], op0=AOp.mult, op1=AOp.mult)
                    for dhalf in range(2):
                        r0 = h * DH + dhalf * 128
                        nc.sync.dma_start(
                            ctxT_d[b, r0:r0 + 128, :], ctx_sbs[dhalf][:])

                # s_vec -> DRAM row (for partition broadcast in final phase)
                nc.sync.dma_start(
                    s_row_d[b, :].rearrange("(t p) -> p t", p=128), s_acc[:])
        with tc.tile_pool(name="fnL", bufs=1) as fin, \
             tc.tile_pool(name="fcL", bufs=4) as finc, \
             tc.tile_pool(name="psFL", bufs=1, space="PSUM") as psF:
            final_phase(BL - 1, fin, finc, psF, wide=True)

    nc.compile()
    return nc


class _Runner:
    """Persistent PJRT runner (mirrors bass2jax.run_bass_via_pjrt, reusable)."""

    def __init__(self, nc, n_cores):
        import jax
        from jax.sharding import Mesh, PartitionSpec
        from jax.experimental.shard_map import shard_map
        import concourse.mybir as mybir
        from concourse import bass2jax
        from concourse.bass2jax import _bass_exec_p, install_neuronx_cc_hook

        install_neuronx_cc_hook()
        self.jax = jax
        self.n_cores = n_cores
        partition_name = (nc.partition_id_tensor.name
                          if nc.partition_id_tensor else None)
        in_names, out_names, out_avals, zero_outs = [], [], [], []
        for alloc in nc.m.functions[0].allocations:
            if not isinstance(alloc, mybir.MemoryLocationSet):
                continue
            name = alloc.memorylocations[0].name
            if alloc.kind == "ExternalInput":
                if name != partition_name:
                    in_names.append(name)
            elif alloc.kind == "ExternalOutput":
                shape = tuple(alloc.tensor_shape)
                dtype = mybir.dt.np(alloc.dtype)
                out_names.append(name)
                out_avals.append(jax.core.ShapedArray(shape, dtype))
                zero_outs.append(np.zeros(shape, dtype))
        self.in_names, self.out_names = in_names, out_names
        self.out_avals, self.zero_outs = out_avals, zero_outs
        n_params = len(in_names)
        self.n_params = n_params
        all_in_names = list(in_names) + list(out_names)
        if partition_name is not None:
            all_in_names.append(partition_name)

        def _body(*args):
            operands = list(args)
            if partition_name is not None:
                operands.append(bass2jax.partition_id_tensor())
            outs = _bass_exec_p.bind(
                *operands,
                out_avals=tuple(out_avals),
                in_names=tuple(all_in_names),
                out_names=tuple(out_names),
                lowering_input_output_aliases=(),
                sim_require_finite=True,
                sim_require_nnan=True,
                nc=nc,
            )
            return tuple(outs)

        devices = jax.devices()[:n_cores]
        mesh = Mesh(np.asarray(devices), ("core",))
        in_specs = (PartitionSpec("core"),) * (n_params + len(out_names))
        out_specs = (PartitionSpec("core"),) * len(out_names)
        self.fn = jax.jit(
            shard_map(_body, mesh=mesh, in_specs=in_specs,
                      out_specs=out_specs, check_rep=False),
            keep_unused=True,
        )
        self._dev_inputs = None

    def put_inputs(self, in_maps):
        per_core = [[np.asarray(m[name]) for name in self.in_names]
                    for m in in_maps]
        arrs = [np.concatenate([per_core[c][i] for c in range(self.n_cores)],
                               axis=0)
                for i in range(self.n_params)]
        self._dev_inputs = [self.jax.device_put(a) for a in arrs]
        self.jax.block_until_ready(self._dev_inputs)

    def run(self):
        zeros = [np.zeros((self.n_cores * z.shape[0], *z.shape[1:]), z.dtype)
                 for z in self.zero_outs]
        out = self.fn(*self._dev_inputs, *zeros)
        self.jax.block_until_ready(out)
        return out

    def results(self, out_arrs):
        return [
            {
                n: np.asarray(out_arrs[i]).reshape(
                    self.n_cores, *self.out_avals[i].shape)[c]
                for i, n in enumerate(self.out_names)
            }
            for c in range(self.n_cores)
        ]


_CACHE = {}


def _get_runner():
    if "runner" not in _CACHE:
        _CACHE["runner"] = _Runner(_build_nc(), NCORES)
    return _CACHE["runner"]


def _prep_inputs(x, mask, in_proj_w, in_proj_b, out_w, out_b):
    x = np.asarray(x, dtype=np.float32)
    mask = np.asarray(mask)
    in_proj_w = np.asarray(in_proj_w, dtype=np.float32)
    in_proj_b = np.asarray(in_proj_b, dtype=np.float32)
    out_w = np.asarray(out_w, dtype=np.float32)
    out_b = np.asarray(out_b, dtype=np.float32)

    wt = np.ascontiguousarray(in_proj_w.T)                            # [E, 3E]
    owt = np.ascontiguousarray(out_w.T)                               # [E, E]
    bqk_v = in_proj_b[:2 * E].copy()
    bqk_v[:E] = bqk_v[:E] / 16.0     # q bias folded into ACT Identity bias
    bqk = np.ascontiguousarray(bqk_v.reshape(16, 128).T)              # [128,16]
    bv = in_proj_b[2 * E:].reshape(1, E).copy()
    outb = (np.float32(S) * out_b).reshape(1, E).copy()
    ones_c = np.ones((128, 1), np.float32)
    ones_r = np.ones((1, 128), np.float32)

    in_maps = []
    for c in range(NCORES):
        bs = slice(c * BL, (c + 1) * BL)
        xTc = np.ascontiguousarray(x[bs].transpose(0, 2, 1))          # [BL,E,S]
        mb = np.where(mask[bs], 0.0, -1e9).astype(np.float32)         # [BL, S]
        mbc = np.ascontiguousarray(
            mb.reshape(BL, NKT, 128).transpose(0, 2, 1))              # [BL,128,NKT]
        in_maps.append({
            "xT": xTc, "wt": wt, "owt": owt, "bqk": bqk, "bv": bv,
            "maskb": mbc, "outb": outb, "ones_c": ones_c, "ones_r": ones_r,
        })
    return in_maps


def kernel(x, mask, in_proj_w, in_proj_b, out_w, out_b):
    r = _get_runner()
    in_maps = _prep_inputs(x, mask, in_proj_w, in_proj_b, out_w, out_b)
    r.put_inputs(in_maps)
    out = r.run()
    res = r.results(out)
    return np.concatenate([res[c]["out_o"] for c in range(NCORES)], axis=0)



# revision 4
# speedup vs baseline: 1.2291x; 1.2291x over previous
"""Trainium2 Bass kernel for nn_CrossAttentionLayer (B=16, S=2048, E=1024, H=4).

Sharding: data-parallel over batch B across 8 NeuronCores (2 batches/core);
projection weights replicated.

Math reduction: the reference's final einsum("bqk,bke->be", avg_w, attn_output)
collapses to
    out[b] = (s_vec @ ctx_full) @ out_w.T + S * out_b
where s_vec[k] = mean_h sum_q softmax_w_h[q, k] and ctx_full [S, E] is the
per-head attention context - the full [S, E] output projection is never built.

v2 layout: all matmul operands bf16 (full PE rate, halves SBUF + DMA), with
in-projection weights fully resident in SBUF for the whole kernel.  Softmax
denominators come from a DVE bf16 add-chain over the exp tiles plus a single
all-ones [128,128] matmul per q-quarter (partition sum + broadcast in one
shot), replacing the per-k-tile M=1 ones matmuls of v1.  Exp tiles are bf16
so the colsum STT runs in the DVE 16-bit fast path.  Final projection stays
fp32r for accuracy.

Per-core pipeline per (batch, head):
  1. kT [128,2,S], v [128,16,DH] generated from resident xT/wt (bf16); biases
     folded into ACT Identity copyback / DVE add.
  2. per sq-quarter: qT via 16 MMs + ACT (scale 1/sqrt(DH), bias); per k-tile:
     scoresT = kT.T@qT (2 MMs), ACT exp with key-padding mask as per-partition
     bias -> bf16 tile, DVE chain add into l_acc, 2 ctx MMs accumulate
     ctxT += v.T @ expT; then ones128-MM sums l_acc over partitions into a
     broadcast PSUM tile, fast reciprocal -> r_bc; per-key colsums c (-> s_vec)
     via fused DVE multiply+accumulate; ctxT written back *1/l as bf16.
  3. u = s_vec @ ctx_full via fused DVE multiply+accumulate over spilled ctxT,
     then out[b] = u @ out_w.T + S*out_b as an M=1 fp32r matmul chain.
"""
import numpy as np
from contextlib import ExitStack

B, S, E, H = 16, 2048, 1024, 4
DH = E // H          # 256
NCORES = 8
BL = B // NCORES     # batches per core
NKT = S // 128       # sk tiles
NET = E // 128       # e tiles
QTR = 512            # sq chunk


def _build_nc(repeat=1):
    import concourse.bass as bass
    import concourse.tile as tile
    from concourse import bacc, mybir

    F32 = mybir.dt.float32
    F32R = mybir.dt.float32r
    BF16 = mybir.dt.bfloat16
    AOp = mybir.AluOpType
    Act = mybir.ActivationFunctionType

    nc = bacc.Bacc()

    xT = nc.dram_tensor("xT", [BL, E, S], BF16, kind="ExternalInput")
    wt = nc.dram_tensor("wt", [E, 3 * E], BF16, kind="ExternalInput")
    owt = nc.dram_tensor("owt", [E, E], F32R, kind="ExternalInput")
    bqk = nc.dram_tensor("bqk", [128, 16], F32, kind="ExternalInput")
    bv = nc.dram_tensor("bv", [1, E], F32, kind="ExternalInput")
    maskb = nc.dram_tensor("maskb", [BL, 128, NKT], F32, kind="ExternalInput")
    outb = nc.dram_tensor("outb", [1, E], F32, kind="ExternalInput")
    ones_t = nc.dram_tensor("ones_t", [128, 128], BF16, kind="ExternalInput")
    out_o = nc.dram_tensor("out_o", [BL, E], F32, kind="ExternalOutput")

    ctxT_d = nc.dram_tensor("ctxT_d", [BL, E, S], BF16)
    s_row_d = nc.dram_tensor("s_row_d", [BL, S], BF16)

    wt_t = wt.rearrange("(eo p) j -> p eo j", p=128)    # [128, 8, 3072]
    owt_t = owt.rearrange("(eo p) j -> p eo j", p=128)  # [128, 8, 1024]

    def bcast_ap(dram_ap, n):
        # read the same n contiguous DRAM elements into all 128 partitions
        return bass.AP(tensor=dram_ap.tensor, offset=dram_ap.offset,
                       ap=[[0, 128], [1, n]])

    with tile.TileContext(nc) as tc, ExitStack() as ctx:
        const = ctx.enter_context(tc.tile_pool(name="const", bufs=1))

        bqk_sb = const.tile([128, 16], F32)
        nc.sync.dma_start(bqk_sb[:], bqk[:, :])
        maskb_sb = const.tile([128, BL * NKT], F32)
        for b in range(BL):
            nc.sync.dma_start(maskb_sb[:, b * NKT:(b + 1) * NKT], maskb[b, :, :])
        ones_sb = const.tile([128, 128], BF16)
        nc.sync.dma_start(ones_sb[:], ones_t[:, :])
        bv_bc = const.tile([128, E], F32)
        nc.sync.dma_start(bv_bc[:], bcast_ap(bv[0, :], E))
        outb_sb = const.tile([1, E], F32)
        nc.sync.dma_start(outb_sb[:], outb[:, :])
        # resident weights: in-projection (bf16) + out-projection (f32r)
        wt_sb = const.tile([128, 8, 3 * E], BF16)
        for j in range(6):
            nc.sync.dma_start(wt_sb[:, :, j * 512:(j + 1) * 512],
                              wt_t[:, :, j * 512:(j + 1) * 512])
        owt_sb = const.tile([128, 8, E], F32R)
        nc.sync.dma_start(owt_sb[:], owt_t[:, :, :])

        def final_phase(b, fin, finc, psF):
            # final reduction for batch b, emitted with pools owned by the
            # following batch's scope so both can stay allocated concurrently
            s_bc = fin.tile([128, S], BF16, tag="s_bc")
            nc.sync.dma_start(s_bc[:], bcast_ap(s_row_d[b, :], S))

            u_cols = [fin.tile([128, NET], F32, tag=f"u_col{i}",
                               name=f"u_col{i}") for i in range(2)]
            dummy2 = fin.tile([128, S // 2], BF16,
                              tag="dummy2")  # reused across both halves
            for et in range(NET):
                for i in range(2):
                    ks = slice(i * (S // 2), (i + 1) * (S // 2))
                    ctx_t = finc.tile([128, S // 2], BF16,
                                      tag="ctx_t", name=f"ctx_t{et}_{i}")
                    nc.sync.dma_start(
                        ctx_t[:], ctxT_d[b, et * 128:(et + 1) * 128, ks])
                    nc.vector.scalar_tensor_tensor(
                        out=dummy2[:], in0=ctx_t[:], scalar=1.0,
                        in1=s_bc[:, ks],
                        op0=AOp.mult, op1=AOp.mult,
                        accum_out=u_cols[i][:, et:et + 1])
            u_r = fin.tile([128, NET], F32R, tag="u_r")
            nc.vector.tensor_add(u_r[:], u_cols[0][:], u_cols[1][:])

            for fc in range(2):
                cs = slice(fc * 512, (fc + 1) * 512)
                psumF = psF.tile([1, 512], F32, tag="psumF",
                                 name=f"psumF{fc}")
                for et in range(NET):
                    nc.tensor.matmul(psumF[:], u_r[:, et:et + 1],
                                     owt_sb[:, et, cs],
                                     start=(et == 0),
                                     stop=(et == NET - 1))
                out_row = fin.tile([1, 512], F32, tag="out_row",
                                   name=f"out_row{fc}")
                nc.vector.tensor_add(out_row[:], psumF[:],
                                     outb_sb[:, cs])
                nc.sync.dma_start(out_o[b:b + 1, cs], out_row[:])

        for rep in range(repeat):
          for b in range(BL):
            with tc.tile_pool(name=f"xp{b}r{rep}", bufs=1) as xp, \
                 tc.tile_pool(name=f"hd{b}r{rep}", bufs=1) as head, \
                 tc.tile_pool(name=f"ep{b}r{rep}", bufs=2) as epool, \
                 tc.tile_pool(name=f"e1{b}r{rep}", bufs=1) as epool1, \
                 tc.tile_pool(name=f"sm{b}r{rep}", bufs=2) as small, \
                 tc.tile_pool(name=f"cx{b}r{rep}", bufs=1) as ctxp, \
                 tc.tile_pool(name=f"ac{b}r{rep}", bufs=1) as acc, \
                 tc.tile_pool(name=f"psB{b}r{rep}", bufs=1, space="PSUM") as psB, \
                 tc.tile_pool(name=f"psS{b}r{rep}", bufs=4, space="PSUM") as psS, \
                 tc.tile_pool(name=f"fn{b}r{rep}", bufs=1) as fin, \
                 tc.tile_pool(name=f"fc{b}r{rep}", bufs=4) as finc, \
                 tc.tile_pool(name=f"psF{b}r{rep}", bufs=1, space="PSUM") as psF:
                if b > 0 or rep > 0:
                    final_phase(b - 1 if b > 0 else BL - 1, fin, finc, psF)
                s_acc = acc.tile([128, NKT], F32, tag="s_acc")
                nc.vector.memset(s_acc[:], 0.0)

                xT_sb = xp.tile([128, 8, S], BF16, tag="xT")
                xr = xT[b].rearrange("(eo p) s -> p eo s", p=128)
                for xc in range(4):
                    nc.sync.dma_start(
                        xT_sb[:, :, xc * 512:(xc + 1) * 512],
                        xr[:, :, xc * 512:(xc + 1) * 512])

                for h in range(H):
                    # ---- generate kT [128, 2, S] and v [128, 16, DH],
                    #      interleaved by xT s-chunk so work starts as soon
                    #      as the first chunk lands ----
                    kTh = head.tile([128, 2, S], BF16, tag="kTh")
                    vh = head.tile([128, NKT, DH], BF16, tag="vh")
                    for sc in range(4):
                        for jt2 in range(2):
                            jcol = 8 + 2 * h + jt2
                            w0 = E + h * DH + jt2 * 128
                            gq = psS.tile([128, 512], F32, tag="sa",
                                          name="gk")
                            for eo in range(8):
                                nc.tensor.matmul(
                                    gq[:],
                                    wt_sb[:, eo, w0:w0 + 128],
                                    xT_sb[:, eo, sc * 512:(sc + 1) * 512],
                                    start=(eo == 0), stop=(eo == 7))
                            nc.scalar.activation(
                                kTh[:, jt2, sc * 512:(sc + 1) * 512], gq[:],
                                Act.Identity,
                                bias=bqk_sb[:, jcol:jcol + 1],
                                scale=1.0)
                        for st in range(4 * sc, 4 * sc + 4):
                            gv = psS.tile([128, DH], F32, tag="sa", name="gv")
                            for eo in range(8):
                                nc.tensor.matmul(
                                    gv[:],
                                    xT_sb[:, eo, st * 128:(st + 1) * 128],
                                    wt_sb[:, eo, 2 * E + h * DH:
                                          2 * E + (h + 1) * DH],
                                    start=(eo == 0), stop=(eo == 7))
                            nc.vector.tensor_add(
                                vh[:, st, :], gv[:],
                                bv_bc[:, h * DH:(h + 1) * DH])

                    # ---- attention over sq quarters ----
                    ctx_sbs = [ctxp.tile([128, S], BF16,
                                         tag=f"ctxsb{d_}",
                                         name=f"ctxsb{d_}") for d_ in range(2)]
                    for qtr in range(S // QTR):
                        q0 = qtr * QTR
                        # generate this quarter's qT [128, 2, QTR]
                        qTh = head.tile([128, 2, QTR], BF16, tag=f"qTh{qtr % 2}",
                                        name="qTh")
                        for jt2 in range(2):
                            jcol = 2 * h + jt2
                            w0 = h * DH + jt2 * 128
                            gq = psS.tile([128, 512], F32, tag="sa", name="gq")
                            for eo in range(8):
                                nc.tensor.matmul(
                                    gq[:],
                                    wt_sb[:, eo, w0:w0 + 128],
                                    xT_sb[:, eo, q0:q0 + QTR],
                                    start=(eo == 0), stop=(eo == 7))
                            nc.scalar.activation(
                                qTh[:, jt2, :], gq[:],
                                Act.Identity,
                                bias=bqk_sb[:, jcol:jcol + 1],
                                scale=1.0 / 16.0)
                        psumC0 = psB.tile([128, QTR], F32, tag="pvc0")
                        psumC1 = psB.tile([128, QTR], F32, tag="pvc1")
                        l_acc = small.tile([128, QTR], BF16, tag="l_acc")
                        expts = []
                        for kt in range(NKT):
                            sa = psS.tile([128, QTR], F32, tag="sa", name="sa")
                            for do in range(2):
                                nc.tensor.matmul(
                                    sa[:],
                                    kTh[:, do, kt * 128:(kt + 1) * 128],
                                    qTh[:, do, :],
                                    start=(do == 0), stop=(do == 1))
                            ep = epool if kt < 8 else epool1
                            et = ep.tile([128, QTR], BF16, tag=f"et{kt}",
                                         name=f"et{kt}")
                            nc.scalar.activation(
                                et[:], sa[:], Act.Exp,
                                bias=maskb_sb[:, b * NKT + kt:b * NKT + kt + 1],
                                scale=1.0)
                            expts.append(et)
                            if kt == 1:
                                nc.vector.tensor_add(
                                    l_acc[:], expts[0][:], expts[1][:])
                            elif kt > 1:
                                nc.vector.tensor_add(
                                    l_acc[:], l_acc[:], et[:])
                            nc.tensor.matmul(
                                psumC0[:], vh[:, kt, 0:128], et[:],
                                start=(kt == 0), stop=(kt == NKT - 1))
                            nc.tensor.matmul(
                                psumC1[:], vh[:, kt, 128:256], et[:],
                                start=(kt == 0), stop=(kt == NKT - 1))
                        # partition-sum l_acc and broadcast to all 128
                        # partitions in one all-ones matmul
                        psumR = psB.tile([128, QTR], F32, tag="psumR",
                                         name="psumR")
                        nc.tensor.matmul(psumR[:], ones_sb[:], l_acc[:],
                                         start=True, stop=True)
                        r_bc = small.tile([128, QTR], F32, tag="r_bc")
                        nc.vector.reciprocal_approx_fast(r_bc[:], psumR[:])
                        r_bc16 = small.tile([128, QTR], BF16, tag="r_bc16")
                        nc.vector.tensor_copy(r_bc16[:], r_bc[:])

                        c_half = small.tile([128, NKT], F32, tag="c_half")
                        dummy = small.tile([128, QTR], BF16, tag="dummy")
                        for kt in range(NKT):
                            nc.vector.scalar_tensor_tensor(
                                out=dummy[:], in0=expts[kt][:],
                                scalar=1.0 / H, in1=r_bc16[:],
                                op0=AOp.mult, op1=AOp.mult,
                                accum_out=c_half[:, kt:kt + 1])
                        nc.vector.tensor_add(s_acc[:], s_acc[:], c_half[:])

                        for dhalf, psC in ((0, psumC0), (1, psumC1)):
                            nc.vector.scalar_tensor_tensor(
                                out=ctx_sbs[dhalf][:, q0:q0 + QTR], in0=psC[:],
                                scalar=1.0,
                                in1=r_bc[:], op0=AOp.mult, op1=AOp.mult)
                    for dhalf in range(2):
                        r0 = h * DH + dhalf * 128
                        nc.sync.dma_start(
                            ctxT_d[b, r0:r0 + 128, :], ctx_sbs[dhalf][:])

                # s_vec -> DRAM row (for partition broadcast in final phase)
                s_b16 = acc.tile([128, NKT], BF16, tag="s_b16")
                nc.vector.tensor_copy(s_b16[:], s_acc[:])
                nc.sync.dma_start(
                    s_row_d[b, :].rearrange("(t p) -> p t", p=128), s_b16[:])
        with tc.tile_pool(name="fnL", bufs=1) as fin, \
             tc.tile_pool(name="fcL", bufs=4) as finc, \
             tc.tile_pool(name="psFL", bufs=1, space="PSUM") as psF:
            final_phase(BL - 1, fin, finc, psF)

    nc.compile()
    return nc


class _Runner:
    """Persistent PJRT runner (mirrors bass2jax.run_bass_via_pjrt, reusable)."""

    def __init__(self, nc, n_cores):
        import jax
        from jax.sharding import Mesh, PartitionSpec
        from jax.experimental.shard_map import shard_map
        import concourse.mybir as mybir
        from concourse import bass2jax
        from concourse.bass2jax import _bass_exec_p, install_neuronx_cc_hook

        install_neuronx_cc_hook()
        self.jax = jax
        self.n_cores = n_cores
        partition_name = (nc.partition_id_tensor.name
                          if nc.partition_id_tensor else None)
        in_names, out_names, out_avals, zero_outs = [], [], [], []
        for alloc in nc.m.functions[0].allocations:
            if not isinstance(alloc, mybir.MemoryLocationSet):
                continue
            name = alloc.memorylocations[0].name
            if alloc.kind == "ExternalInput":
                if name != partition_name:
                    in_names.append(name)
            elif alloc.kind == "ExternalOutput":
                shape = tuple(alloc.tensor_shape)
                dtype = mybir.dt.np(alloc.dtype)
                out_names.append(name)
                out_avals.append(jax.core.ShapedArray(shape, dtype))
                zero_outs.append(np.zeros(shape, dtype))
        self.in_names, self.out_names = in_names, out_names
        self.out_avals, self.zero_outs = out_avals, zero_outs
        n_params = len(in_names)
        self.n_params = n_params
        all_in_names = list(in_names) + list(out_names)
        if partition_name is not None:
            all_in_names.append(partition_name)

        def _body(*args):
            operands = list(args)
            if partition_name is not None:
                operands.append(bass2jax.partition_id_tensor())
            outs = _bass_exec_p.bind(
                *operands,
                out_avals=tuple(out_avals),
                in_names=tuple(all_in_names),
                out_names=tuple(out_names),
                lowering_input_output_aliases=(),
                sim_require_finite=True,
                sim_require_nnan=True,
                nc=nc,
            )
            return tuple(outs)

        devices = jax.devices()[:n_cores]
        mesh = Mesh(np.asarray(devices), ("core",))
        in_specs = (PartitionSpec("core"),) * (n_params + len(out_names))
        out_specs = (PartitionSpec("core"),) * len(out_names)
        self.fn = jax.jit(
            shard_map(_body, mesh=mesh, in_specs=in_specs,
                      out_specs=out_specs, check_rep=False),
            keep_unused=True,
        )
        self._dev_inputs = None

    def put_inputs(self, in_maps):
        per_core = [[np.asarray(m[name]) for name in self.in_names]
                    for m in in_maps]
        arrs = [np.concatenate([per_core[c][i] for c in range(self.n_cores)],
                               axis=0)
                for i in range(self.n_params)]
        self._dev_inputs = [self.jax.device_put(a) for a in arrs]
        self.jax.block_until_ready(self._dev_inputs)

    def run(self):
        zeros = [np.zeros((self.n_cores * z.shape[0], *z.shape[1:]), z.dtype)
                 for z in self.zero_outs]
        out = self.fn(*self._dev_inputs, *zeros)
        self.jax.block_until_ready(out)
        return out

    def results(self, out_arrs):
        return [
            {
                n: np.asarray(out_arrs[i]).reshape(
                    self.n_cores, *self.out_avals[i].shape)[c]
                for i, n in enumerate(self.out_names)
            }
            for c in range(self.n_cores)
        ]


_CACHE = {}


def _get_runner():
    if "runner" not in _CACHE:
        _CACHE["runner"] = _Runner(_build_nc(), NCORES)
    return _CACHE["runner"]


def _prep_inputs(x, mask, in_proj_w, in_proj_b, out_w, out_b):
    import ml_dtypes
    BF = ml_dtypes.bfloat16
    x = np.asarray(x, dtype=np.float32)
    mask = np.asarray(mask)
    in_proj_w = np.asarray(in_proj_w, dtype=np.float32)
    in_proj_b = np.asarray(in_proj_b, dtype=np.float32)
    out_w = np.asarray(out_w, dtype=np.float32)
    out_b = np.asarray(out_b, dtype=np.float32)

    x_bf = x.astype(BF)                                               # [B,S,E]
    wt = np.ascontiguousarray(in_proj_w.T.astype(BF))                 # [E, 3E]
    owt = np.ascontiguousarray(out_w.T)                               # [E, E]
    bqk_v = in_proj_b[:2 * E].copy()
    bqk_v[:E] = bqk_v[:E] / 16.0     # q bias folded into ACT Identity bias
    bqk = np.ascontiguousarray(bqk_v.reshape(16, 128).T)              # [128,16]
    bv = in_proj_b[2 * E:].reshape(1, E).copy()
    outb = (np.float32(S) * out_b).reshape(1, E).copy()
    ones_t = np.ones((128, 128), BF)

    in_maps = []
    for c in range(NCORES):
        bs = slice(c * BL, (c + 1) * BL)
        xTc = np.ascontiguousarray(x_bf[bs].transpose(0, 2, 1))       # [BL,E,S]
        mb = np.where(mask[bs], 0.0, -1e9).astype(np.float32)         # [BL, S]
        mbc = np.ascontiguousarray(
            mb.reshape(BL, NKT, 128).transpose(0, 2, 1))              # [BL,128,NKT]
        in_maps.append({
            "xT": xTc, "wt": wt, "owt": owt, "bqk": bqk, "bv": bv,
            "maskb": mbc, "outb": outb, "ones_t": ones_t,
        })
    return in_maps


def kernel(x, mask, in_proj_w, in_proj_b, out_w, out_b):
    r = _get_runner()
    in_maps = _prep_inputs(x, mask, in_proj_w, in_proj_b, out_w, out_b)
    r.put_inputs(in_maps)
    out = r.run()
    res = r.results(out)
    return np.concatenate([res[c]["out_o"] for c in range(NCORES)], axis=0)
